# revision 48
# baseline (speedup 1.0000x reference)
"""MultiHeadedAttention Trainium2 kernel (v2).

Problem: B=2, S=4096, d_model=512, H=8 heads, dk=64.
  q/k/v proj -> scaled dot-product attention per head -> concat -> out proj.

Sharding: 8 cores = (batch b in {0,1}) x (head-pair hp in {0..3}).
Each core computes, for its batch and its 2 heads (all fp16 operands,
fp32 PSUM accumulation):
  QpT/KpT/VpT = (x @ W[:, hp*128:hp*128+128] + b).T        [128, 4096] fp16
  Vp          = V in key-major layout via DMA XBAR transpose, with a ones
                column per head (softmax denominator comes from it)
  S^T[k, q]   = Kp Qp^T per head (k-major scores, fp32 PSUM)
  P^T         = exp(S^T / 8)  split across three engines:
                  ACT: exact exp activation
                  DVE/Pool: averaged-pair Schraudolph bit-trick exp
                    i16 = trunc(s*(2^10/ln2)/8 + 15317); p ~ fp16(i16)
                    + fp16(i16+512)*2^-0.5   (halved-grid average, ~1.6%
                    pointwise, cancels further through the softmax ratio)
  ctx         = q-major context: out[q, d] (+ Z column) accumulated over
                key blocks with pt as the (free) stationary operand --
                65-wide matmuls instead of 512-wide k-major ones
  ctx_n       = ctx * (1/Z) per-partition scalars (DVE), then DMA XBAR
                transpose -> ctxT [d, q]
  outT_part   = Wo^T-chunks @ ctxT -> [512, 4096] fp16 (transposed)
Host: out[b] = sum_hp(outT_partial).T + bo.
"""

import numpy as np

import concourse.bass as bass
import concourse.bacc as bacc
import concourse.mybir as mybir
import concourse.tile as tile
from concourse.bass_utils import run_bass_kernel_spmd
from concourse.masks import make_identity

F32 = mybir.dt.float32
F16 = mybir.dt.float16
I16 = mybir.dt.int16
EXP = mybir.ActivationFunctionType.Exp
IDENT = mybir.ActivationFunctionType.Identity
MUL = mybir.AluOpType.mult
ADD = mybir.AluOpType.add

B = 2
S = 4096
D = 512           # d_model
H = 8
DK = 64
HP = 4            # head pairs per batch
DL = 128          # local channels per core (2 heads)
CJ = 4            # contraction chunks of 128 over d_model
QT = S // 512     # 8 q-tiles of 512
KB = S // 128     # 32 k-blocks of 128
SCALE = 1.0 / np.sqrt(DK).item()  # 1/8
CTX_LAG = 10

# exp split: per (qt, kb) head-0's [128, 512] score tile goes to ACT
# (exact exp); head-1's [128, 512] gets a PAIR of Schraudolph bit-trick
# samples (i16 and i16+512).  The pair average -- exp(x) ~ fp16(i16) +
# 2^-0.5 * fp16(i16+512) -- is folded into the ctx matmul by
# accumulating both samples, the second against a 2^-0.5-scaled copy of
# V.  The softmax ratio cancels the overall scale.  GPSIMD cannot touch
# PSUM, so DVE computes all first samples (PSUM reads) while the int16
# +512 second samples (SBUF->SBUF) are split Pool | DVE.
ADD_DVE = 0       # head-1 +512 columns computed on DVE (rest on Pool)
# Schraudolph constants (fp16 bit layout), scale 1/8 folded into the slope
SCH_A = (1024.0 / np.log(2.0)).item() * SCALE
SCH_B = 15.0 * 1024.0 - 43.0
SCH_W = (2.0 ** -0.5)

TRACE = False
LAST_RESULTS = None
USE_SCHRAUDOLPH = True   # bisect flag: False -> exact ACT exp for head 1 too

_prog_cache = {}


def _emit(nc, reps=1):
    xqT = nc.declare_dram_parameter("xqT", [D, S], F16, isOutput=False)
    xkT = nc.declare_dram_parameter("xkT", [D, S], F16, isOutput=False)
    xvT = nc.declare_dram_parameter("xvT", [D, S], F16, isOutput=False)
    wq = nc.declare_dram_parameter("wq", [D, DL], F16, isOutput=False)
    wk = nc.declare_dram_parameter("wk", [D, DL], F16, isOutput=False)
    wv = nc.declare_dram_parameter("wv", [D, DL], F16, isOutput=False)
    bq = nc.declare_dram_parameter("bq", [DL, 1], F32, isOutput=False)
    bk = nc.declare_dram_parameter("bk", [DL, 1], F32, isOutput=False)
    bv = nc.declare_dram_parameter("bv", [DL, 1], F32, isOutput=False)
    wo = nc.declare_dram_parameter("wo", [DL, D], F16, isOutput=False)
    outT = nc.declare_dram_parameter("outT", [D, S], F16, isOutput=True)

    with tile.TileContext(nc) as tc:
        with (
            nc.allow_low_precision(reason="fp16 matmuls + schraudolph exp"),
            tc.tile_pool(name="singles", bufs=1) as singles,
            tc.tile_pool(name="xin", bufs=6) as xin,
            tc.tile_pool(name="proj", bufs=1) as proj,
            tc.tile_pool(name="pt", bufs=16) as ptpool,
            tc.tile_pool(name="sad", bufs=16) as sadpool,
            tc.tile_pool(name="ctxn", bufs=2) as ctxnpool,
            tc.tile_pool(name="ctxT", bufs=2) as ctxTpool,
            tc.tile_pool(name="outp", bufs=4) as outpool,
            tc.tile_pool(name="norm", bufs=4) as normpool,
            tc.tile_pool(name="mm512", bufs=2, space="PSUM") as mmps,
            tc.tile_pool(name="spsa", bufs=2, space="PSUM") as spsumA,
            tc.tile_pool(name="spsb", bufs=2, space="PSUM") as spsumB,
            tc.tile_pool(name="cps", bufs=2, space="PSUM") as cpsum,
        ):
            # --- constants / weights ---
            ident = singles.tile([128, 128], F32, tag="ident")
            make_identity(nc, ident)
            warmsrc = singles.tile([1, 8], F32, tag="warmsrc")
            nc.vector.memset(warmsrc, 1.0)
            warm = singles.tile([1, 8], F32, tag="warm")
            nc.scalar.activation(warm, warmsrc, EXP, scale=1.0)

            w_sb = {}
            b_sb = {}

            def load_w(name, w, bias):
                t = singles.tile([128, CJ, DL], F16, tag=name)
                nc.sync.dma_start(
                    out=t, in_=w[:].rearrange("(j p) d -> p j d", p=128)
                )
                w_sb[name] = t
                bt = singles.tile([DL, 1], F32, tag="b" + name[1])
                nc.sync.dma_start(out=bt, in_=bias[:])
                b_sb["b" + name[1]] = bt

            # --- projections: dst = (x @ W + b).T, channel-major [128, S] ---
            qpT = proj.tile([DL, S], F16, tag="qpT")
            kpT = proj.tile([DL, S], F16, tag="kpT")
            vpT = proj.tile([DL, S], F32, tag="vpT")

            # V in key-major layout: [key 128, kb 32, 130] where per head h
            # cols [65h, 65h+64) are channels and col 65h+64 is ones (the
            # softmax denominator comes out of the ctx matmul for free).
            # vp_w is head-1's slice scaled by 2^-0.5 for the second
            # Schraudolph sample's ctx accumulation.
            vp = proj.tile([128, KB, 130], F16, tag="vp")
            nc.vector.memset(vp, 1.0)
            vp_w = proj.tile([128, KB, 65], F16, tag="vp_w")

            def project_st(xT, wname, dst, st, evac_eng):
                """Columns [st*512, st*512+512) of dst = (x @ W + b).T"""
                c0 = st * 512
                xTr = xT[:].rearrange("(j p) s -> p j s", p=128)
                xt = xin.tile([128, CJ, 512], F16, tag="xin")
                nc.sync.dma_start(out=xt, in_=xTr[:, :, c0 : c0 + 512])
                ps = mmps.tile([128, 512], F32, tag="mm512")
                for cj in range(CJ):
                    nc.tensor.matmul(
                        ps,
                        lhsT=w_sb[wname][:, cj, :],
                        rhs=xt[:, cj, :],
                        start=(cj == 0),
                        stop=(cj == CJ - 1),
                    )
                bias = b_sb["b" + wname[1]]
                if evac_eng == "act":
                    nc.scalar.activation(
                        dst[:, c0 : c0 + 512], ps, IDENT, bias=bias, scale=1.0
                    )
                else:
                    nc.vector.tensor_scalar_add(dst[:, c0 : c0 + 512], ps, bias)

            def v_transpose_st(st):
                """Vp key-major blocks for the 4 k-blocks of one s-tile
                (PE transpose via identity, then per-head evac copies)."""
                ks = slice(st * 4, (st + 1) * 4)
                for kb in range(st * 4, (st + 1) * 4):
                    tp = mmps.tile([128, 512], F32, tag="mm512")
                    nc.tensor.transpose(
                        tp[:, 0:128], vpT[:, kb * 128 : (kb + 1) * 128], ident
                    )
                    nc.scalar.copy(vp[:, kb, 0:64], tp[:, 0:64])
                    nc.vector.tensor_copy(vp[:, kb, 65:129], tp[:, 64:128])
                nc.gpsimd.tensor_scalar(
                    out=vp_w[:, ks, :],
                    in0=vp[:, ks, 65:130],
                    scalar1=SCH_W,
                    scalar2=None,
                    op0=MUL,
                )

            # Prologue: weights + the projections needed before slot (0, 0).
            # Everything else is interleaved into the attention slot stream.
            load_w("wq", wq, bq)
            project_st(xqT, "wq", qpT, 0, "act")
            load_w("wk", wk, bk)
            load_w("wv", wv, bv)
            wo_sb = singles.tile([DL, D], F16, tag="wo")
            nc.sync.dma_start(out=wo_sb, in_=wo[:])
            project_st(xkT, "wk", kpT, 0, "act")
            project_st(xvT, "wv", vpT, 0, "dve")
            v_transpose_st(0)
            project_st(xkT, "wk", kpT, 1, "act")
            project_st(xvT, "wv", vpT, 1, "dve")
            v_transpose_st(1)

            # --- attention + output projection, per q-tile of 512 ---
            # Epilogue work (normalize + transpose + Wo proj) for q-tile qt
            # is emitted piecewise during q-tile qt+1's kb loop.
            state = {}

            def norm_recip(qt, cps_h):
                zs = normpool.tile([128, 2, 4], F32, tag="zs", name="zs")
                rec = normpool.tile([128, 2, 4], F32, tag="rec", name="rec")
                state["rec"] = rec
                for h in (0, 1):
                    nc.vector.tensor_copy(zs[:, h, :], cps_h[h][:, :, 64])
                nc.vector.reciprocal(rec[:, :, :], zs[:, :, :])

            def norm_mul_qs(qt, cps_h, qs):
                # GPSIMD cannot read PSUM; DVE multiplies by 1/Z via the
                # per-partition tensor_scalar operand.
                if qs == 0:
                    state["ctxn"] = ctxnpool.tile(
                        [128, 4, 128], F32, tag="ctxn", name="ctxn"
                    )
                for h in (0, 1):
                    nc.vector.tensor_scalar(
                        out=state["ctxn"][:, qs, 64 * h : 64 * h + 64],
                        in0=cps_h[h][:, qs, 0:64],
                        scalar1=state["rec"][:, h, qs : qs + 1],
                        scalar2=None,
                        op0=MUL,
                    )

            def ctxt_qs(qt, qs):
                if qs == 0:
                    state["ctxT"] = ctxTpool.tile(
                        [128, 512], F16, tag="ctxT", name="ctxT"
                    )
                tp = mmps.tile([128, 512], F32, tag="mm512")
                nc.tensor.transpose(tp[:, 0:128], state["ctxn"][:, qs, :], ident)
                if qs % 2 == 0:
                    nc.scalar.copy(
                        state["ctxT"][:, qs * 128 : (qs + 1) * 128], tp[:, 0:128]
                    )
                else:
                    nc.vector.tensor_copy(
                        state["ctxT"][:, qs * 128 : (qs + 1) * 128], tp[:, 0:128]
                    )

            def oproj_j(qt, j):
                qsl = slice(qt * 512, (qt + 1) * 512)
                ops = mmps.tile([128, 512], F32, tag="mm512")
                nc.tensor.matmul(
                    ops,
                    lhsT=wo_sb[:, j * 128 : (j + 1) * 128],
                    rhs=state["ctxT"],
                    start=True,
                    stop=True,
                )
                ot = outpool.tile([128, 512], F16, tag="out")
                nc.scalar.copy(ot, ops)
                nc.sync.dma_start(out=outT[j * 128 : (j + 1) * 128, qsl], in_=ot)

            def epilogue_step(step, qt, cps_h):
                if step == 11:
                    norm_recip(qt, cps_h)
                elif 12 <= step <= 15:
                    norm_mul_qs(qt, cps_h, step - 12)
                elif 16 <= step <= 19:
                    ctxt_qs(qt, step - 16)
                elif 20 <= step <= 23:
                    oproj_j(qt, step - 20)

            def scores_exp(qt, kb):
                qsl = slice(qt * 512, (qt + 1) * 512)
                spa = spsumA.tile([128, 512], F32, tag="spsa")
                spb = spsumB.tile([128, 512], F32, tag="spsb")
                for h, sp in ((0, spa), (1, spb)):
                    nc.tensor.matmul(
                        sp,
                        lhsT=kpT[h * 64 : (h + 1) * 64, kb * 128 : (kb + 1) * 128],
                        rhs=qpT[h * 64 : (h + 1) * 64, qsl],
                        start=True,
                        stop=True,
                    )
                # head 0: exact exp on ACT
                pt = ptpool.tile([128, 512], F16, tag="pt")
                nc.scalar.activation(pt, spa, EXP, scale=SCALE)
                # head 1: two Schraudolph samples (i16, i16+512)
                sad = sadpool.tile([128, 2, 512], I16, tag="sad")
                if USE_SCHRAUDOLPH:
                    nc.vector.tensor_scalar(
                        out=sad[:, 0, :],
                        in0=spb,
                        scalar1=SCH_A,
                        scalar2=SCH_B,
                        op0=MUL,
                        op1=ADD,
                    )
                    if ADD_DVE > 0:
                        nc.vector.tensor_scalar_add(
                            sad[:, 1, 0:ADD_DVE], sad[:, 0, 0:ADD_DVE], 512.0
                        )
                    nc.gpsimd.tensor_scalar_add(
                        sad[:, 1, ADD_DVE:512], sad[:, 0, ADD_DVE:512], 512.0
                    )
                else:
                    nc.scalar.activation(
                        sad.bitcast(F16)[:, 0, :], spb, EXP, scale=SCALE
                    )
                    nc.vector.memset(sad[:, 1, :], 0)
                return (pt, sad)

            def ctx_mm(cps_h, kb, ptt):
                pt, sad = ptt
                # One PSUM accumulation group per head per q-tile (a PSUM
                # zero region is a whole bank): start only on the very first
                # matmul, stop on the very last.
                # head 0: plain accumulation of the exact-exp tile
                for qs in range(4):
                    nc.tensor.matmul(
                        cps_h[0][:, qs, 0:65],
                        lhsT=pt[:, qs * 128 : (qs + 1) * 128],
                        rhs=vp[:, kb, 0:65],
                        start=(kb == 0 and qs == 0),
                        stop=(kb == KB - 1 and qs == 3),
                        skip_group_check=True,
                    )
                # head 1: both Schraudolph samples; sample B against the
                # 2^-0.5-scaled V copy (pair-average folded into the sum)
                for qs in range(4):
                    blk = lambda smp: sad.bitcast(F16)[
                        :, smp, qs * 128 : (qs + 1) * 128
                    ]
                    for smp, rhs in ((0, vp[:, kb, 65:130]), (1, vp_w[:, kb, :])):
                        nc.tensor.matmul(
                            cps_h[1][:, qs, 0:65],
                            lhsT=blk(smp),
                            rhs=rhs,
                            start=(kb == 0 and qs == 0 and smp == 0),
                            stop=(kb == KB - 1 and qs == 3 and smp == 1),
                            skip_group_check=True,
                        )

            # Flat software pipeline over all (qt, kb) slots: ctx lags
            # CTX_LAG slots globally (crossing q-tile boundaries).  The
            # first 4 ctx batches of each q-tile are deferred to slot
            # kb=14 so the previous q-tile's normalize (slots 11-13) has
            # released the cps buffers before the start=True write.  The
            # remaining K/V/Q projections stream through early slots so
            # the PE never sees a serial projection phase.
            slots = [
                (qt, kb)
                for _ in range(reps)
                for qt in range(QT)
                for kb in range(KB)
            ]
            nslots = len(slots)
            cps_of = {}
            pts = {}
            deferred = {}
            pending = None  # (qt, cps_h) awaiting epilogue

            def emit_ctx(j):
                jqt, jkb = slots[j]
                if jkb == 0:
                    cps0 = cpsum.tile([128, 4, 128], F32, tag="cps", name="cps0")
                    cps1 = cpsum.tile([128, 4, 128], F32, tag="cps", name="cps1")
                    cps_of[jqt] = (cps0, cps1)
                ctx_mm(cps_of[jqt], jkb, pts.pop(j))

            for i, (qt, kb) in enumerate(slots):
                j = i - CTX_LAG
                if kb == 14 and qt in deferred:
                    for j2 in deferred.pop(qt):
                        emit_ctx(j2)
                if j >= 0:
                    jqt, jkb = slots[j]
                    if jkb < 4:
                        deferred.setdefault(jqt, []).append(j)
                    else:
                        emit_ctx(j)
                pts[i] = scores_exp(qt, kb)
                # interleaved projections (first pass only), two s-tiles
                # of lookahead so the evac/transpose latency is covered
                if qt == 0 and kb % 4 == 0 and 1 < (st := kb // 4 + 2) < QT:
                    project_st(xkT, "wk", kpT, st, "act")
                    project_st(xvT, "wv", vpT, st, "dve")
                    v_transpose_st(st)
                if kb == 20 and qt + 1 < QT:
                    project_st(xqT, "wq", qpT, qt + 1, "dve")
                if pending is not None:
                    epilogue_step(kb, *pending)
                if kb == KB - 1:
                    pending = (qt, cps_of[qt])
            for j in range(nslots - CTX_LAG, nslots):
                emit_ctx(j)
            for step in range(11, 24):
                epilogue_step(step, *pending)
    return nc


def _build(reps=1):
    if reps not in _prog_cache:
        nc = bacc.Bacc()
        _emit(nc, reps)
        nc.compile()
        _prog_cache[reps] = nc
    return _prog_cache[reps]


def _make_in_maps(query, key, value, Wq, bq, Wk, bk, Wv, bv, Wo):
    f16 = lambda a: np.ascontiguousarray(a, dtype=np.float16)
    f32 = lambda a: np.ascontiguousarray(a, dtype=np.float32)
    in_maps = []
    for b in range(B):
        xqT = f16(query[b].T)
        xkT = f16(key[b].T)
        xvT = f16(value[b].T)
        for hp in range(HP):
            cs = slice(hp * DL, (hp + 1) * DL)
            in_maps.append(
                {
                    "xqT": xqT,
                    "xkT": xkT,
                    "xvT": xvT,
                    "wq": f16(Wq[:, cs]),
                    "wk": f16(Wk[:, cs]),
                    "wv": f16(Wv[:, cs]),
                    "bq": f32(bq[cs].reshape(DL, 1)),
                    "bk": f32(bk[cs].reshape(DL, 1)),
                    "bv": f32(bv[cs].reshape(DL, 1)),
                    "wo": f16(Wo[cs, :]),
                }
            )
    return in_maps


def kernel(query, key, value, Wq, bq, Wk, bk, Wv, bv, Wo, bo):
    global LAST_RESULTS
    query = np.asarray(query, dtype=np.float32)
    key = np.asarray(key, dtype=np.float32)
    value = np.asarray(value, dtype=np.float32)
    Wq = np.asarray(Wq, dtype=np.float32)
    Wk = np.asarray(Wk, dtype=np.float32)
    Wv = np.asarray(Wv, dtype=np.float32)
    Wo = np.asarray(Wo, dtype=np.float32)
    bq = np.asarray(bq, dtype=np.float32)
    bk = np.asarray(bk, dtype=np.float32)
    bv = np.asarray(bv, dtype=np.float32)
    bo = np.asarray(bo, dtype=np.float32)

    nc = _build()
    in_maps = _make_in_maps(query, key, value, Wq, bq, Wk, bk, Wv, bv, Wo)

    res = run_bass_kernel_spmd(nc, in_maps, list(range(B * HP)), trace=TRACE)
    LAST_RESULTS = res

    out = np.empty((B, S, D), dtype=np.float32)
    for b in range(B):
        acc = res.results[b * HP]["outT"].astype(np.float32)
        for hp in range(1, HP):
            acc = acc + res.results[b * HP + hp]["outT"].astype(np.float32)
        out[b] = acc.T + bo
    return out


# revision 53
# speedup vs baseline: 1.0238x; 1.0238x over previous
"""MultiHeadedAttention Trainium2 kernel (v3).

Problem: B=2, S=4096, d_model=512, H=8 heads, dk=64.
  q/k/v proj -> scaled dot-product attention per head -> concat -> out proj.

Sharding: 8 cores = (batch b in {0,1}) x (head-pair hp in {0..3}).
Each core computes, for its batch and its 2 heads (fp16 operands,
fp32 PSUM accumulation):
  QpT/KpT     = (x @ W[:, hp*128:+128] + b).T  [128, 4096] fp16; VpT f32
  Vp          = V in key-major layout via PE transpose + identity, with a
                ones column per head (softmax denominator falls out of
                the ctx matmul); vp_w = 2^-0.5 * Vp head-1 slice
  S^T[k, q]   = Kp Qp^T per head (k-major scores, fp32 PSUM)
  P^T         = exp(S^T / 8):
                  head 0: exact exp on ACT
                  head 1: two Schraudolph bit-trick samples
                    i16 = trunc(s*(2^10/ln2)/8 + 15317)  (DVE, from PSUM)
                    i16+512                              (Pool, SBUF int16)
                    pair average exp ~ fp16(i16) + 2^-0.5*fp16(i16+512)
                    is folded into ctx by accumulating both samples, the
                    second against vp_w; the softmax ratio cancels the
                    overall scale (~4e-3 end-to-end error)
  ctx         = q-major context: out[q, d | Z] accumulated over key
                blocks with pt as the stationary operand -- 65-wide
                matmuls instead of 512-wide k-major ones (half PE cost);
                one PSUM accumulation group per bank (start/stop only on
                the first/last matmul -- hw zero-region semantics)
  ctx_n       = ctx * (1/Z) per-partition scalars (DVE), PE transpose
                -> ctxT [d, q] fp16
  outT_part   = Wo^T-chunks @ ctxT -> [512, 4096] fp16 (transposed)
Host: out[b] = sum_hp(outT_partial).T + bo.

The emission is one flat software-pipelined slot stream over all
(qt, kb): ctx lags CTX_LAG slots (crossing q-tile boundaries, with the
first 4 ctx batches of each q-tile deferred past the previous tile's
normalize), remaining K/V/Q projections are interleaved into early
slots, and each q-tile's epilogue is paced through slots 11..23 of the
next tile.  Engine queues are strictly in-order, so emission order is
chosen so an instruction's dependencies are (nearly) always satisfied
when it reaches the head of its queue.
"""

import numpy as np

import concourse.bass as bass
import concourse.bacc as bacc
import concourse.mybir as mybir
import concourse.tile as tile
from concourse.bass_utils import run_bass_kernel_spmd
from concourse.masks import make_identity

F32 = mybir.dt.float32
F16 = mybir.dt.float16
I16 = mybir.dt.int16
EXP = mybir.ActivationFunctionType.Exp
IDENT = mybir.ActivationFunctionType.Identity
MUL = mybir.AluOpType.mult
ADD = mybir.AluOpType.add

B = 2
S = 4096
D = 512           # d_model
H = 8
DK = 64
HP = 4            # head pairs per batch
DL = 128          # local channels per core (2 heads)
CJ = 4            # contraction chunks of 128 over d_model
QT = S // 512     # 8 q-tiles of 512
KB = S // 128     # 32 k-blocks of 128
SCALE = 1.0 / np.sqrt(DK).item()  # 1/8
CTX_LAG = 10

# exp split: per (qt, kb) head-0's [128, 512] score tile goes to ACT
# (exact exp); head-1's [128, 512] gets a PAIR of Schraudolph bit-trick
# samples (i16 and i16+512).  The pair average -- exp(x) ~ fp16(i16) +
# 2^-0.5 * fp16(i16+512) -- is folded into the ctx matmul by
# accumulating both samples, the second against a 2^-0.5-scaled copy of
# V.  The softmax ratio cancels the overall scale.  GPSIMD cannot touch
# PSUM, so DVE computes all first samples (PSUM reads) while the int16
# +512 second samples (SBUF->SBUF) are split Pool | DVE.
ADD_DVE = 0       # head-1 +512 columns computed on DVE (rest on Pool)
# Schraudolph constants (fp16 bit layout), scale 1/8 folded into the slope
SCH_A = (1024.0 / np.log(2.0)).item() * SCALE
SCH_B = 15.0 * 1024.0 - 43.0
SCH_W = (2.0 ** -0.5)

TRACE = False
LAST_RESULTS = None
USE_SCHRAUDOLPH = True   # bisect flag: False -> exact ACT exp for head 1 too

_prog_cache = {}


def _emit(nc, reps=1):
    xqT = nc.declare_dram_parameter("xqT", [D, S], F16, isOutput=False)
    xkT = nc.declare_dram_parameter("xkT", [D, S], F16, isOutput=False)
    xvT = nc.declare_dram_parameter("xvT", [D, S], F16, isOutput=False)
    wq = nc.declare_dram_parameter("wq", [D, DL], F16, isOutput=False)
    wk = nc.declare_dram_parameter("wk", [D, DL], F16, isOutput=False)
    wv = nc.declare_dram_parameter("wv", [D, DL], F16, isOutput=False)
    bq = nc.declare_dram_parameter("bq", [DL, 1], F32, isOutput=False)
    bk = nc.declare_dram_parameter("bk", [DL, 1], F32, isOutput=False)
    bv = nc.declare_dram_parameter("bv", [DL, 1], F32, isOutput=False)
    wo = nc.declare_dram_parameter("wo", [DL, D], F16, isOutput=False)
    outT = nc.declare_dram_parameter("outT", [D, S], F16, isOutput=True)

    with tile.TileContext(nc) as tc:
        with (
            nc.allow_low_precision(reason="fp16 matmuls + schraudolph exp"),
            tc.tile_pool(name="singles", bufs=1) as singles,
            tc.tile_pool(name="xin", bufs=6) as xin,
            tc.tile_pool(name="proj", bufs=1) as proj,
            tc.tile_pool(name="pt", bufs=16) as ptpool,
            tc.tile_pool(name="sad", bufs=16) as sadpool,
            tc.tile_pool(name="ctxn", bufs=2) as ctxnpool,
            tc.tile_pool(name="ctxT", bufs=2) as ctxTpool,
            tc.tile_pool(name="outp", bufs=4) as outpool,
            tc.tile_pool(name="norm", bufs=4) as normpool,
            tc.tile_pool(name="mm512", bufs=2, space="PSUM") as mmps,
            tc.tile_pool(name="spsa", bufs=2, space="PSUM") as spsumA,
            tc.tile_pool(name="spsb", bufs=2, space="PSUM") as spsumB,
            tc.tile_pool(name="cps", bufs=2, space="PSUM") as cpsum,
        ):
            # --- constants / weights ---
            ident = singles.tile([128, 128], F16, tag="ident")
            make_identity(nc, ident)
            warmsrc = singles.tile([1, 8], F32, tag="warmsrc")
            nc.vector.memset(warmsrc, 1.0)
            warm = singles.tile([1, 8], F32, tag="warm")
            nc.scalar.activation(warm, warmsrc, EXP, scale=1.0)

            w_sb = {}
            b_sb = {}

            def load_w(name, w, bias):
                t = singles.tile([128, CJ, DL], F16, tag=name)
                nc.sync.dma_start(
                    out=t, in_=w[:].rearrange("(j p) d -> p j d", p=128)
                )
                w_sb[name] = t
                bt = singles.tile([DL, 1], F32, tag="b" + name[1])
                nc.sync.dma_start(out=bt, in_=bias[:])
                b_sb["b" + name[1]] = bt

            # --- projections: dst = (x @ W + b).T, channel-major [128, S] ---
            qpT = proj.tile([DL, S], F16, tag="qpT")
            kpT = proj.tile([DL, S], F16, tag="kpT")
            vpT = proj.tile([DL, S], F16, tag="vpT")

            # V in key-major layout: [key 128, kb 32, 130] where per head h
            # cols [65h, 65h+64) are channels and col 65h+64 is ones (the
            # softmax denominator comes out of the ctx matmul for free).
            # vp_w is head-1's slice scaled by 2^-0.5 for the second
            # Schraudolph sample's ctx accumulation.
            vp = proj.tile([128, KB, 130], F16, tag="vp")
            nc.vector.memset(vp, 1.0)
            vp_w = proj.tile([128, KB, 65], F16, tag="vp_w")

            def project_st(xT, wname, dst, st, evac_eng):
                """Columns [st*512, st*512+512) of dst = (x @ W + b).T"""
                c0 = st * 512
                xTr = xT[:].rearrange("(j p) s -> p j s", p=128)
                xt = xin.tile([128, CJ, 512], F16, tag="xin")
                nc.sync.dma_start(out=xt, in_=xTr[:, :, c0 : c0 + 512])
                ps = mmps.tile([128, 512], F32, tag="mm512")
                for cj in range(CJ):
                    nc.tensor.matmul(
                        ps,
                        lhsT=w_sb[wname][:, cj, :],
                        rhs=xt[:, cj, :],
                        start=(cj == 0),
                        stop=(cj == CJ - 1),
                    )
                bias = b_sb["b" + wname[1]]
                if evac_eng == "act":
                    nc.scalar.activation(
                        dst[:, c0 : c0 + 512], ps, IDENT, bias=bias, scale=1.0
                    )
                else:
                    nc.vector.tensor_scalar_add(dst[:, c0 : c0 + 512], ps, bias)

            def v_transpose_st(st):
                """Vp key-major blocks for the 4 k-blocks of one s-tile
                (PE transpose via identity, then per-head evac copies)."""
                ks = slice(st * 4, (st + 1) * 4)
                for kb in range(st * 4, (st + 1) * 4):
                    tp = mmps.tile([128, 512], F32, tag="mm512")
                    tpf = tp.bitcast(F16)
                    nc.tensor.transpose(
                        tpf[:, 0:128], vpT[:, kb * 128 : (kb + 1) * 128], ident
                    )
                    nc.scalar.copy(vp[:, kb, 0:64], tpf[:, 0:64])
                    nc.vector.tensor_copy(vp[:, kb, 65:129], tpf[:, 64:128])
                nc.gpsimd.tensor_scalar(
                    out=vp_w[:, ks, :],
                    in0=vp[:, ks, 65:130],
                    scalar1=SCH_W,
                    scalar2=None,
                    op0=MUL,
                )

            # Prologue: weights + the projections needed before slot (0, 0).
            # Everything else is interleaved into the attention slot stream.
            load_w("wq", wq, bq)
            project_st(xqT, "wq", qpT, 0, "act")
            load_w("wk", wk, bk)
            load_w("wv", wv, bv)
            wo_sb = singles.tile([DL, D], F16, tag="wo")
            nc.sync.dma_start(out=wo_sb, in_=wo[:])
            project_st(xkT, "wk", kpT, 0, "act")
            project_st(xvT, "wv", vpT, 0, "dve")
            v_transpose_st(0)
            project_st(xkT, "wk", kpT, 1, "act")
            project_st(xvT, "wv", vpT, 1, "dve")
            v_transpose_st(1)

            # --- attention + output projection, per q-tile of 512 ---
            # Epilogue work (normalize + transpose + Wo proj) for q-tile qt
            # is emitted piecewise during q-tile qt+1's kb loop.
            state = {}

            def norm_recip(qt, cps_h):
                zs = normpool.tile([128, 2, 4], F32, tag="zs", name="zs")
                rec = normpool.tile([128, 2, 4], F32, tag="rec", name="rec")
                state["rec"] = rec
                for h in (0, 1):
                    nc.vector.tensor_copy(zs[:, h, :], cps_h[h][:, :, 64])
                nc.vector.reciprocal(rec[:, :, :], zs[:, :, :])

            def norm_mul_h(qt, cps_h, h):
                # GPSIMD cannot read PSUM; DVE multiplies by 1/Z via the
                # per-partition tensor_scalar operand (two qs per op call).
                if h == 0:
                    state["ctxn"] = ctxnpool.tile(
                        [128, 4, 128], F16, tag="ctxn", name="ctxn"
                    )
                for qs in range(4):
                    nc.vector.tensor_scalar(
                        out=state["ctxn"][:, qs, 64 * h : 64 * h + 64],
                        in0=cps_h[h][:, qs, 0:64],
                        scalar1=state["rec"][:, h, qs : qs + 1],
                        scalar2=None,
                        op0=MUL,
                    )

            def ctxt_qs(qt, qs):
                if qs == 0:
                    state["ctxT"] = ctxTpool.tile(
                        [128, 512], F16, tag="ctxT", name="ctxT"
                    )
                tp = mmps.tile([128, 512], F32, tag="mm512")
                tpf = tp.bitcast(F16)
                nc.tensor.transpose(tpf[:, 0:128], state["ctxn"][:, qs, :], ident)
                if qs % 2 == 0:
                    nc.scalar.copy(
                        state["ctxT"][:, qs * 128 : (qs + 1) * 128], tpf[:, 0:128]
                    )
                else:
                    nc.vector.tensor_copy(
                        state["ctxT"][:, qs * 128 : (qs + 1) * 128], tpf[:, 0:128]
                    )

            def oproj_j(qt, j):
                qsl = slice(qt * 512, (qt + 1) * 512)
                ops = mmps.tile([128, 512], F32, tag="mm512")
                nc.tensor.matmul(
                    ops,
                    lhsT=wo_sb[:, j * 128 : (j + 1) * 128],
                    rhs=state["ctxT"],
                    start=True,
                    stop=True,
                )
                ot = outpool.tile([128, 512], F16, tag="out")
                nc.scalar.copy(ot, ops)
                nc.sync.dma_start(out=outT[j * 128 : (j + 1) * 128, qsl], in_=ot)

            def epilogue_step(step, qt, cps_h):
                if step == 11:
                    norm_recip(qt, cps_h)
                elif step in (12, 13):
                    norm_mul_h(qt, cps_h, step - 12)
                elif 14 <= step <= 17:
                    ctxt_qs(qt, step - 14)
                elif 18 <= step <= 21:
                    oproj_j(qt, step - 18)

            def scores_exp(qt, kb):
                qsl = slice(qt * 512, (qt + 1) * 512)
                spa = spsumA.tile([128, 512], F32, tag="spsa")
                spb = spsumB.tile([128, 512], F32, tag="spsb")
                for h, sp in ((0, spa), (1, spb)):
                    nc.tensor.matmul(
                        sp,
                        lhsT=kpT[h * 64 : (h + 1) * 64, kb * 128 : (kb + 1) * 128],
                        rhs=qpT[h * 64 : (h + 1) * 64, qsl],
                        start=True,
                        stop=True,
                    )
                # head 0: exact exp on ACT
                pt = ptpool.tile([128, 512], F16, tag="pt")
                nc.scalar.activation(pt, spa, EXP, scale=SCALE)
                # head 1: two Schraudolph samples (i16, i16+512)
                sad = sadpool.tile([128, 2, 512], I16, tag="sad")
                if USE_SCHRAUDOLPH:
                    nc.vector.tensor_scalar(
                        out=sad[:, 0, :],
                        in0=spb,
                        scalar1=SCH_A,
                        scalar2=SCH_B,
                        op0=MUL,
                        op1=ADD,
                    )
                    if ADD_DVE > 0:
                        nc.vector.tensor_scalar_add(
                            sad[:, 1, 0:ADD_DVE], sad[:, 0, 0:ADD_DVE], 512.0
                        )
                    nc.gpsimd.tensor_scalar_add(
                        sad[:, 1, ADD_DVE:512], sad[:, 0, ADD_DVE:512], 512.0
                    )
                else:
                    nc.scalar.activation(
                        sad.bitcast(F16)[:, 0, :], spb, EXP, scale=SCALE
                    )
                    nc.vector.memset(sad[:, 1, :], 0)
                return (pt, sad)

            def ctx_mm(cps_h, kb, ptt):
                pt, sad = ptt
                # One PSUM accumulation group per head per q-tile (a PSUM
                # zero region is a whole bank): start only on the very first
                # matmul, stop on the very last.
                # head 0: plain accumulation of the exact-exp tile
                for qs in range(4):
                    nc.tensor.matmul(
                        cps_h[0][:, qs, 0:65],
                        lhsT=pt[:, qs * 128 : (qs + 1) * 128],
                        rhs=vp[:, kb, 0:65],
                        start=(kb == 0 and qs == 0),
                        stop=(kb == KB - 1 and qs == 3),
                        skip_group_check=True,
                    )
                # head 1: both Schraudolph samples; sample B against the
                # 2^-0.5-scaled V copy (pair-average folded into the sum)
                for qs in range(4):
                    blk = lambda smp: sad.bitcast(F16)[
                        :, smp, qs * 128 : (qs + 1) * 128
                    ]
                    for smp, rhs in ((0, vp[:, kb, 65:130]), (1, vp_w[:, kb, :])):
                        nc.tensor.matmul(
                            cps_h[1][:, qs, 0:65],
                            lhsT=blk(smp),
                            rhs=rhs,
                            start=(kb == 0 and qs == 0 and smp == 0),
                            stop=(kb == KB - 1 and qs == 3 and smp == 1),
                            skip_group_check=True,
                        )

            # Flat software pipeline over all (qt, kb) slots: ctx lags
            # CTX_LAG slots globally (crossing q-tile boundaries).  The
            # first 4 ctx batches of each q-tile are deferred to slot
            # kb=14 so the previous q-tile's normalize (slots 11-13) has
            # released the cps buffers before the start=True write.  The
            # remaining K/V/Q projections stream through early slots so
            # the PE never sees a serial projection phase.
            slots = [
                (qt, kb)
                for _ in range(reps)
                for qt in range(QT)
                for kb in range(KB)
            ]
            nslots = len(slots)
            cps_of = {}
            pts = {}
            deferred = {}
            pending = None  # (qt, cps_h) awaiting epilogue

            def emit_ctx(j):
                jqt, jkb = slots[j]
                if jkb == 0:
                    cps0 = cpsum.tile([128, 4, 128], F32, tag="cps", name="cps0")
                    cps1 = cpsum.tile([128, 4, 128], F32, tag="cps", name="cps1")
                    cps_of[jqt] = (cps0, cps1)
                ctx_mm(cps_of[jqt], jkb, pts.pop(j))

            for i, (qt, kb) in enumerate(slots):
                j = i - CTX_LAG
                if kb == 14 and qt in deferred:
                    for j2 in deferred.pop(qt):
                        emit_ctx(j2)
                if j >= 0:
                    jqt, jkb = slots[j]
                    if jkb < 4:
                        deferred.setdefault(jqt, []).append(j)
                    else:
                        emit_ctx(j)
                pts[i] = scores_exp(qt, kb)
                # interleaved projections (first pass only), two s-tiles
                # of lookahead so the evac/transpose latency is covered
                if qt == 0 and kb % 4 == 0 and 1 < (st := kb // 4 + 2) < QT:
                    project_st(xkT, "wk", kpT, st, "act")
                    project_st(xvT, "wv", vpT, st, "dve")
                    v_transpose_st(st)
                if kb == 20 and qt + 1 < QT:
                    project_st(xqT, "wq", qpT, qt + 1, "dve")
                if pending is not None:
                    epilogue_step(kb, *pending)
                if kb == KB - 1:
                    pending = (qt, cps_of[qt])
            for j in range(nslots - CTX_LAG, nslots):
                emit_ctx(j)
            for step in range(11, 22):
                epilogue_step(step, *pending)
    return nc


def _build(reps=1):
    if reps not in _prog_cache:
        nc = bacc.Bacc()
        _emit(nc, reps)
        nc.compile()
        _prog_cache[reps] = nc
    return _prog_cache[reps]


def _make_in_maps(query, key, value, Wq, bq, Wk, bk, Wv, bv, Wo):
    f16 = lambda a: np.ascontiguousarray(a, dtype=np.float16)
    f32 = lambda a: np.ascontiguousarray(a, dtype=np.float32)
    in_maps = []
    for b in range(B):
        xqT = f16(query[b].T)
        xkT = f16(key[b].T)
        xvT = f16(value[b].T)
        for hp in range(HP):
            cs = slice(hp * DL, (hp + 1) * DL)
            in_maps.append(
                {
                    "xqT": xqT,
                    "xkT": xkT,
                    "xvT": xvT,
                    "wq": f16(Wq[:, cs]),
                    "wk": f16(Wk[:, cs]),
                    "wv": f16(Wv[:, cs]),
                    "bq": f32(bq[cs].reshape(DL, 1)),
                    "bk": f32(bk[cs].reshape(DL, 1)),
                    "bv": f32(bv[cs].reshape(DL, 1)),
                    "wo": f16(Wo[cs, :]),
                }
            )
    return in_maps


def kernel(query, key, value, Wq, bq, Wk, bk, Wv, bv, Wo, bo):
    global LAST_RESULTS
    query = np.asarray(query, dtype=np.float32)
    key = np.asarray(key, dtype=np.float32)
    value = np.asarray(value, dtype=np.float32)
    Wq = np.asarray(Wq, dtype=np.float32)
    Wk = np.asarray(Wk, dtype=np.float32)
    Wv = np.asarray(Wv, dtype=np.float32)
    Wo = np.asarray(Wo, dtype=np.float32)
    bq = np.asarray(bq, dtype=np.float32)
    bk = np.asarray(bk, dtype=np.float32)
    bv = np.asarray(bv, dtype=np.float32)
    bo = np.asarray(bo, dtype=np.float32)

    nc = _build()
    in_maps = _make_in_maps(query, key, value, Wq, bq, Wk, bk, Wv, bv, Wo)

    res = run_bass_kernel_spmd(nc, in_maps, list(range(B * HP)), trace=TRACE)
    LAST_RESULTS = res

    out = np.empty((B, S, D), dtype=np.float32)
    for b in range(B):
        acc = res.results[b * HP]["outT"].astype(np.float32)
        for hp in range(1, HP):
            acc = acc + res.results[b * HP + hp]["outT"].astype(np.float32)
        out[b] = acc.T + bo
    return out


# revision 56
# speedup vs baseline: 1.0283x; 1.0044x over previous
"""MultiHeadedAttention Trainium2 kernel (v3).

Problem: B=2, S=4096, d_model=512, H=8 heads, dk=64.
  q/k/v proj -> scaled dot-product attention per head -> concat -> out proj.

Sharding: 8 cores = (batch b in {0,1}) x (head-pair hp in {0..3}).
Each core computes, for its batch and its 2 heads (fp16 operands,
fp32 PSUM accumulation):
  QpT/KpT     = (x @ W[:, hp*128:+128] + b).T  [128, 4096] fp16; VpT f32
  Vp          = V in key-major layout via PE transpose + identity, with a
                ones column per head (softmax denominator falls out of
                the ctx matmul); vp_w = 2^-0.5 * Vp head-1 slice
  S^T[k, q]   = Kp Qp^T per head (k-major scores, fp32 PSUM)
  P^T         = exp(S^T / 8):
                  head 0: exact exp on ACT
                  head 1: two Schraudolph bit-trick samples
                    i16 = trunc(s*(2^10/ln2)/8 + 15317)  (DVE, from PSUM)
                    i16+512                              (Pool, SBUF int16)
                    pair average exp ~ fp16(i16) + 2^-0.5*fp16(i16+512)
                    is folded into ctx by accumulating both samples, the
                    second against vp_w; the softmax ratio cancels the
                    overall scale (~4e-3 end-to-end error)
  ctx         = q-major context: out[q, d | Z] accumulated over key
                blocks with pt as the stationary operand -- 65-wide
                matmuls instead of 512-wide k-major ones (half PE cost);
                one PSUM accumulation group per bank (start/stop only on
                the first/last matmul -- hw zero-region semantics)
  ctx_n       = ctx * (1/Z) per-partition scalars (DVE), PE transpose
                -> ctxT [d, q] fp16
  outT_part   = Wo^T-chunks @ ctxT -> [512, 4096] fp16 (transposed)
Host: out[b] = sum_hp(outT_partial).T + bo.

The emission is one flat software-pipelined slot stream over all
(qt, kb): ctx lags CTX_LAG slots (crossing q-tile boundaries, with the
first 4 ctx batches of each q-tile deferred past the previous tile's
normalize), remaining K/V/Q projections are interleaved into early
slots, and each q-tile's epilogue is paced through slots 11..23 of the
next tile.  Engine queues are strictly in-order, so emission order is
chosen so an instruction's dependencies are (nearly) always satisfied
when it reaches the head of its queue.
"""

import numpy as np

import concourse.bass as bass
import concourse.bacc as bacc
import concourse.mybir as mybir
import concourse.tile as tile
from concourse.bass_utils import run_bass_kernel_spmd
from concourse.masks import make_identity

F32 = mybir.dt.float32
F16 = mybir.dt.float16
I16 = mybir.dt.int16
EXP = mybir.ActivationFunctionType.Exp
IDENT = mybir.ActivationFunctionType.Identity
MUL = mybir.AluOpType.mult
ADD = mybir.AluOpType.add

B = 2
S = 4096
D = 512           # d_model
H = 8
DK = 64
HP = 4            # head pairs per batch
DL = 128          # local channels per core (2 heads)
CJ = 4            # contraction chunks of 128 over d_model
QT = S // 512     # 8 q-tiles of 512
KB = S // 128     # 32 k-blocks of 128
SCALE = 1.0 / np.sqrt(DK).item()  # 1/8
CTX_LAG = 10

# exp split: per (qt, kb) head-0's [128, 512] score tile goes to ACT
# (exact exp); head-1's [128, 512] gets a PAIR of Schraudolph bit-trick
# samples (i16 and i16+512).  The pair average -- exp(x) ~ fp16(i16) +
# 2^-0.5 * fp16(i16+512) -- is folded into the ctx matmul by
# accumulating both samples, the second against a 2^-0.5-scaled copy of
# V.  The softmax ratio cancels the overall scale.  GPSIMD cannot touch
# PSUM, so DVE computes all first samples (PSUM reads) while the int16
# +512 second samples (SBUF->SBUF) are split Pool | DVE.
ADD_DVE = 0       # head-1 +512 columns computed on DVE (rest on Pool)
# Schraudolph constants (fp16 bit layout), scale 1/8 folded into the slope
SCH_A = (1024.0 / np.log(2.0)).item() * SCALE
SCH_B = 15.0 * 1024.0 - 43.0
SCH_W = (2.0 ** -0.5)

TRACE = False
LAST_RESULTS = None
USE_SCHRAUDOLPH = True   # bisect flag: False -> exact ACT exp for head 1 too

_prog_cache = {}


def _emit(nc, reps=1):
    xqT = nc.declare_dram_parameter("xqT", [D, S], F16, isOutput=False)
    xkT = nc.declare_dram_parameter("xkT", [D, S], F16, isOutput=False)
    xvT = nc.declare_dram_parameter("xvT", [D, S], F16, isOutput=False)
    wq = nc.declare_dram_parameter("wq", [D, DL], F16, isOutput=False)
    wk = nc.declare_dram_parameter("wk", [D, DL], F16, isOutput=False)
    wv = nc.declare_dram_parameter("wv", [D, DL], F16, isOutput=False)
    bq = nc.declare_dram_parameter("bq", [DL, 1], F32, isOutput=False)
    bk = nc.declare_dram_parameter("bk", [DL, 1], F32, isOutput=False)
    bv = nc.declare_dram_parameter("bv", [DL, 1], F32, isOutput=False)
    wo = nc.declare_dram_parameter("wo", [DL, D], F16, isOutput=False)
    outT = nc.declare_dram_parameter("outT", [D, S], F16, isOutput=True)

    with tile.TileContext(nc) as tc:
        with (
            nc.allow_low_precision(reason="fp16 matmuls + schraudolph exp"),
            tc.tile_pool(name="singles", bufs=1) as singles,
            tc.tile_pool(name="xin", bufs=8) as xin,
            tc.tile_pool(name="proj", bufs=1) as proj,
            tc.tile_pool(name="pt", bufs=16) as ptpool,
            tc.tile_pool(name="sad", bufs=16) as sadpool,
            tc.tile_pool(name="ctxn", bufs=2) as ctxnpool,
            tc.tile_pool(name="ctxT", bufs=2) as ctxTpool,
            tc.tile_pool(name="outp", bufs=4) as outpool,
            tc.tile_pool(name="norm", bufs=4) as normpool,
            tc.tile_pool(name="mm512", bufs=2, space="PSUM") as mmps,
            tc.tile_pool(name="spsa", bufs=2, space="PSUM") as spsumA,
            tc.tile_pool(name="spsb", bufs=2, space="PSUM") as spsumB,
            tc.tile_pool(name="cps", bufs=2, space="PSUM") as cpsum,
        ):
            # --- constants / weights ---
            ident = singles.tile([128, 128], F16, tag="ident")
            make_identity(nc, ident)
            warmsrc = singles.tile([1, 8], F32, tag="warmsrc")
            nc.vector.memset(warmsrc, 1.0)
            warm = singles.tile([1, 8], F32, tag="warm")
            nc.scalar.activation(warm, warmsrc, EXP, scale=1.0)

            w_sb = {}
            b_sb = {}

            def load_w(name, w, bias):
                t = singles.tile([128, CJ, DL], F16, tag=name)
                nc.sync.dma_start(
                    out=t, in_=w[:].rearrange("(j p) d -> p j d", p=128)
                )
                w_sb[name] = t
                bt = singles.tile([DL, 1], F32, tag="b" + name[1])
                nc.sync.dma_start(out=bt, in_=bias[:])
                b_sb["b" + name[1]] = bt

            # --- projections: dst = (x @ W + b).T, channel-major [128, S] ---
            qpT = proj.tile([DL, S], F16, tag="qpT")
            kpT = proj.tile([DL, S], F16, tag="kpT")
            vpT = proj.tile([DL, S], F16, tag="vpT")

            # V in key-major layout: [key 128, kb 32, 130] where per head h
            # cols [65h, 65h+64) are channels and col 65h+64 is ones (the
            # softmax denominator comes out of the ctx matmul for free).
            # vp_w is head-1's slice scaled by 2^-0.5 for the second
            # Schraudolph sample's ctx accumulation.
            vp = proj.tile([128, KB, 130], F16, tag="vp")
            nc.vector.memset(vp, 1.0)
            vp_w = proj.tile([128, KB, 65], F16, tag="vp_w")

            def project_st(xT, wname, dst, st, evac_eng):
                """Columns [st*512, st*512+512) of dst = (x @ W + b).T"""
                c0 = st * 512
                xTr = xT[:].rearrange("(j p) s -> p j s", p=128)
                xt = xin.tile([128, CJ, 512], F16, tag="xin")
                nc.sync.dma_start(out=xt, in_=xTr[:, :, c0 : c0 + 512])
                ps = mmps.tile([128, 512], F32, tag="mm512")
                for cj in range(CJ):
                    nc.tensor.matmul(
                        ps,
                        lhsT=w_sb[wname][:, cj, :],
                        rhs=xt[:, cj, :],
                        start=(cj == 0),
                        stop=(cj == CJ - 1),
                    )
                bias = b_sb["b" + wname[1]]
                if evac_eng == "act":
                    nc.scalar.activation(
                        dst[:, c0 : c0 + 512], ps, IDENT, bias=bias, scale=1.0
                    )
                else:
                    nc.vector.tensor_scalar_add(dst[:, c0 : c0 + 512], ps, bias)

            def v_transpose_st(st):
                """Vp key-major blocks for the 4 k-blocks of one s-tile
                (PE transpose via identity, then per-head evac copies)."""
                ks = slice(st * 4, (st + 1) * 4)
                for kb in range(st * 4, (st + 1) * 4):
                    tp = mmps.tile([128, 512], F32, tag="mm512")
                    tpf = tp.bitcast(F16)
                    nc.tensor.transpose(
                        tpf[:, 0:128], vpT[:, kb * 128 : (kb + 1) * 128], ident
                    )
                    nc.scalar.copy(vp[:, kb, 0:64], tpf[:, 0:64])
                    nc.vector.tensor_copy(vp[:, kb, 65:129], tpf[:, 64:128])
                nc.gpsimd.tensor_scalar(
                    out=vp_w[:, ks, :],
                    in0=vp[:, ks, 65:130],
                    scalar1=SCH_W,
                    scalar2=None,
                    op0=MUL,
                )

            # Prologue: weights + the projections needed before slot (0, 0).
            # Everything else is interleaved into the attention slot stream.
            load_w("wq", wq, bq)
            project_st(xqT, "wq", qpT, 0, "act")
            load_w("wk", wk, bk)
            load_w("wv", wv, bv)
            wo_sb = singles.tile([DL, D], F16, tag="wo")
            nc.sync.dma_start(out=wo_sb, in_=wo[:])
            project_st(xkT, "wk", kpT, 0, "act")
            project_st(xvT, "wv", vpT, 0, "dve")
            v_transpose_st(0)
            project_st(xkT, "wk", kpT, 1, "act")
            project_st(xvT, "wv", vpT, 1, "dve")
            v_transpose_st(1)

            # --- attention + output projection, per q-tile of 512 ---
            # Epilogue work (normalize + transpose + Wo proj) for q-tile qt
            # is emitted piecewise during q-tile qt+1's kb loop.
            state = {}

            def norm_recip(qt, cps_h):
                zs = normpool.tile([128, 2, 4], F32, tag="zs", name="zs")
                rec = normpool.tile([128, 2, 4], F32, tag="rec", name="rec")
                state["rec"] = rec
                for h in (0, 1):
                    nc.vector.tensor_copy(zs[:, h, :], cps_h[h][:, :, 64])
                nc.vector.reciprocal(rec[:, :, :], zs[:, :, :])

            def norm_mul_h(qt, cps_h, h):
                # GPSIMD cannot read PSUM; DVE multiplies by 1/Z via the
                # per-partition tensor_scalar operand (two qs per op call).
                if h == 0:
                    state["ctxn"] = ctxnpool.tile(
                        [128, 4, 128], F16, tag="ctxn", name="ctxn"
                    )
                for qs in range(4):
                    nc.vector.tensor_scalar(
                        out=state["ctxn"][:, qs, 64 * h : 64 * h + 64],
                        in0=cps_h[h][:, qs, 0:64],
                        scalar1=state["rec"][:, h, qs : qs + 1],
                        scalar2=None,
                        op0=MUL,
                    )

            def ctxt_qs(qt, qs):
                if qs == 0:
                    state["ctxT"] = ctxTpool.tile(
                        [128, 512], F16, tag="ctxT", name="ctxT"
                    )
                tp = mmps.tile([128, 512], F32, tag="mm512")
                tpf = tp.bitcast(F16)
                nc.tensor.transpose(tpf[:, 0:128], state["ctxn"][:, qs, :], ident)
                nc.scalar.copy(
                    state["ctxT"][:, qs * 128 : (qs + 1) * 128], tpf[:, 0:128]
                )

            def oproj_j(qt, j):
                qsl = slice(qt * 512, (qt + 1) * 512)
                ops = mmps.tile([128, 512], F32, tag="mm512")
                nc.tensor.matmul(
                    ops,
                    lhsT=wo_sb[:, j * 128 : (j + 1) * 128],
                    rhs=state["ctxT"],
                    start=True,
                    stop=True,
                )
                ot = outpool.tile([128, 512], F16, tag="out")
                nc.scalar.copy(ot, ops)
                nc.sync.dma_start(out=outT[j * 128 : (j + 1) * 128, qsl], in_=ot)

            def epilogue_step(step, qt, cps_h):
                if step == 11:
                    norm_recip(qt, cps_h)
                elif step in (12, 13):
                    norm_mul_h(qt, cps_h, step - 12)
                elif 14 <= step <= 17:
                    ctxt_qs(qt, step - 14)
                elif 18 <= step <= 21:
                    oproj_j(qt, step - 18)

            def scores_exp(qt, kb):
                qsl = slice(qt * 512, (qt + 1) * 512)
                spa = spsumA.tile([128, 512], F32, tag="spsa")
                spb = spsumB.tile([128, 512], F32, tag="spsb")
                for h, sp in ((0, spa), (1, spb)):
                    nc.tensor.matmul(
                        sp,
                        lhsT=kpT[h * 64 : (h + 1) * 64, kb * 128 : (kb + 1) * 128],
                        rhs=qpT[h * 64 : (h + 1) * 64, qsl],
                        start=True,
                        stop=True,
                    )
                # head 0: exact exp on ACT
                pt = ptpool.tile([128, 512], F16, tag="pt")
                nc.scalar.activation(pt, spa, EXP, scale=SCALE)
                # head 1: two Schraudolph samples (i16, i16+512)
                sad = sadpool.tile([128, 2, 512], I16, tag="sad")
                if USE_SCHRAUDOLPH:
                    nc.vector.tensor_scalar(
                        out=sad[:, 0, :],
                        in0=spb,
                        scalar1=SCH_A,
                        scalar2=SCH_B,
                        op0=MUL,
                        op1=ADD,
                    )
                    if ADD_DVE > 0:
                        nc.vector.tensor_scalar_add(
                            sad[:, 1, 0:ADD_DVE], sad[:, 0, 0:ADD_DVE], 512.0
                        )
                    nc.gpsimd.tensor_scalar_add(
                        sad[:, 1, ADD_DVE:512], sad[:, 0, ADD_DVE:512], 512.0
                    )
                else:
                    nc.scalar.activation(
                        sad.bitcast(F16)[:, 0, :], spb, EXP, scale=SCALE
                    )
                    nc.vector.memset(sad[:, 1, :], 0)
                return (pt, sad)

            def ctx_mm(cps_h, kb, ptt):
                pt, sad = ptt
                # One PSUM accumulation group per head per q-tile (a PSUM
                # zero region is a whole bank): start only on the very first
                # matmul, stop on the very last.
                # head 0: plain accumulation of the exact-exp tile
                for qs in range(4):
                    nc.tensor.matmul(
                        cps_h[0][:, qs, 0:65],
                        lhsT=pt[:, qs * 128 : (qs + 1) * 128],
                        rhs=vp[:, kb, 0:65],
                        start=(kb == 0 and qs == 0),
                        stop=(kb == KB - 1 and qs == 3),
                        skip_group_check=True,
                    )
                # head 1: both Schraudolph samples; sample B against the
                # 2^-0.5-scaled V copy (pair-average folded into the sum)
                for qs in range(4):
                    blk = lambda smp: sad.bitcast(F16)[
                        :, smp, qs * 128 : (qs + 1) * 128
                    ]
                    for smp, rhs in ((0, vp[:, kb, 65:130]), (1, vp_w[:, kb, :])):
                        nc.tensor.matmul(
                            cps_h[1][:, qs, 0:65],
                            lhsT=blk(smp),
                            rhs=rhs,
                            start=(kb == 0 and qs == 0 and smp == 0),
                            stop=(kb == KB - 1 and qs == 3 and smp == 1),
                            skip_group_check=True,
                        )

            # Flat software pipeline over all (qt, kb) slots: ctx lags
            # CTX_LAG slots globally (crossing q-tile boundaries).  The
            # first 4 ctx batches of each q-tile are deferred to slot
            # kb=14 so the previous q-tile's normalize (slots 11-13) has
            # released the cps buffers before the start=True write.  The
            # remaining K/V/Q projections stream through early slots so
            # the PE never sees a serial projection phase.
            slots = [
                (qt, kb)
                for _ in range(reps)
                for qt in range(QT)
                for kb in range(KB)
            ]
            nslots = len(slots)
            cps_of = {}
            pts = {}
            deferred = {}
            pending = None  # (qt, cps_h) awaiting epilogue

            def emit_ctx(j):
                jqt, jkb = slots[j]
                if jkb == 0:
                    cps0 = cpsum.tile([128, 4, 128], F32, tag="cps", name="cps0")
                    cps1 = cpsum.tile([128, 4, 128], F32, tag="cps", name="cps1")
                    cps_of[jqt] = (cps0, cps1)
                ctx_mm(cps_of[jqt], jkb, pts.pop(j))

            for i, (qt, kb) in enumerate(slots):
                j = i - CTX_LAG
                if kb == 14 and qt in deferred:
                    for j2 in deferred.pop(qt):
                        emit_ctx(j2)
                if j >= 0:
                    jqt, jkb = slots[j]
                    if jkb < 4:
                        deferred.setdefault(jqt, []).append(j)
                    else:
                        emit_ctx(j)
                pts[i] = scores_exp(qt, kb)
                # interleaved projections (first pass only), two s-tiles
                # of lookahead so the evac/transpose latency is covered
                if qt == 0 and kb % 4 == 0 and 1 < (st := kb // 4 + 2) < QT:
                    project_st(xkT, "wk", kpT, st, "act")
                    project_st(xvT, "wv", vpT, st, "dve")
                    v_transpose_st(st)
                if kb == 20 and qt + 1 < QT:
                    project_st(xqT, "wq", qpT, qt + 1, "dve")
                if pending is not None:
                    epilogue_step(kb, *pending)
                if kb == KB - 1:
                    pending = (qt, cps_of[qt])
            for j in range(nslots - CTX_LAG, nslots):
                emit_ctx(j)
            for step in range(11, 22):
                epilogue_step(step, *pending)
    return nc


def _build(reps=1):
    if reps not in _prog_cache:
        nc = bacc.Bacc()
        _emit(nc, reps)
        nc.compile()
        _prog_cache[reps] = nc
    return _prog_cache[reps]


def _make_in_maps(query, key, value, Wq, bq, Wk, bk, Wv, bv, Wo):
    f16 = lambda a: np.ascontiguousarray(a, dtype=np.float16)
    f32 = lambda a: np.ascontiguousarray(a, dtype=np.float32)
    in_maps = []
    for b in range(B):
        xqT = f16(query[b].T)
        xkT = f16(key[b].T)
        xvT = f16(value[b].T)
        for hp in range(HP):
            cs = slice(hp * DL, (hp + 1) * DL)
            in_maps.append(
                {
                    "xqT": xqT,
                    "xkT": xkT,
                    "xvT": xvT,
                    "wq": f16(Wq[:, cs]),
                    "wk": f16(Wk[:, cs]),
                    "wv": f16(Wv[:, cs]),
                    "bq": f32(bq[cs].reshape(DL, 1)),
                    "bk": f32(bk[cs].reshape(DL, 1)),
                    "bv": f32(bv[cs].reshape(DL, 1)),
                    "wo": f16(Wo[cs, :]),
                }
            )
    return in_maps


def kernel(query, key, value, Wq, bq, Wk, bk, Wv, bv, Wo, bo):
    global LAST_RESULTS
    query = np.asarray(query, dtype=np.float32)
    key = np.asarray(key, dtype=np.float32)
    value = np.asarray(value, dtype=np.float32)
    Wq = np.asarray(Wq, dtype=np.float32)
    Wk = np.asarray(Wk, dtype=np.float32)
    Wv = np.asarray(Wv, dtype=np.float32)
    Wo = np.asarray(Wo, dtype=np.float32)
    bq = np.asarray(bq, dtype=np.float32)
    bk = np.asarray(bk, dtype=np.float32)
    bv = np.asarray(bv, dtype=np.float32)
    bo = np.asarray(bo, dtype=np.float32)

    nc = _build()
    in_maps = _make_in_maps(query, key, value, Wq, bq, Wk, bk, Wv, bv, Wo)

    res = run_bass_kernel_spmd(nc, in_maps, list(range(B * HP)), trace=TRACE)
    LAST_RESULTS = res

    out = np.empty((B, S, D), dtype=np.float32)
    for b in range(B):
        acc = res.results[b * HP]["outT"].astype(np.float32)
        for hp in range(1, HP):
            acc = acc + res.results[b * HP + hp]["outT"].astype(np.float32)
        out[b] = acc.T + bo
    return out


# revision 71
# speedup vs baseline: 1.0397x; 1.0111x over previous
"""MultiHeadedAttention Trainium2 kernel (v3).

Problem: B=2, S=4096, d_model=512, H=8 heads, dk=64.
  q/k/v proj -> scaled dot-product attention per head -> concat -> out proj.

Sharding: 8 cores = (batch b in {0,1}) x (head-pair hp in {0..3}).
Each core computes, for its batch and its 2 heads (fp16 operands,
fp32 PSUM accumulation):
  QpT/KpT     = (x @ W[:, hp*128:+128] + b).T  [128, 4096] fp16; VpT f32
  Vp          = V in key-major layout via PE transpose + identity, with a
                ones column per head (softmax denominator falls out of
                the ctx matmul); vp_w = 2^-0.5 * Vp head-1 slice
  S^T[k, q]   = Kp Qp^T per head (k-major scores, fp32 PSUM)
  P^T         = exp(S^T / 8):
                  head 0: exact exp on ACT
                  head 1: two Schraudolph bit-trick samples
                    i16 = trunc(s*(2^10/ln2)/8 + 15317)  (DVE, from PSUM)
                    i16+512                              (Pool, SBUF int16)
                    pair average exp ~ fp16(i16) + 2^-0.5*fp16(i16+512)
                    is folded into ctx by accumulating both samples, the
                    second against vp_w; the softmax ratio cancels the
                    overall scale (~4e-3 end-to-end error)
  ctx         = q-major context: out[q, d | Z] accumulated over key
                blocks with pt as the stationary operand -- 65-wide
                matmuls instead of 512-wide k-major ones (half PE cost);
                one PSUM accumulation group per bank (start/stop only on
                the first/last matmul -- hw zero-region semantics)
  ctx_n       = ctx * (1/Z) per-partition scalars (DVE), PE transpose
                -> ctxT [d, q] fp16
  outT_part   = Wo^T-chunks @ ctxT -> [512, 4096] fp16 (transposed)
Host: out[b] = sum_hp(outT_partial).T + bo.

The emission is one flat software-pipelined slot stream over all
(qt, kb): ctx lags CTX_LAG slots (crossing q-tile boundaries, with the
first 4 ctx batches of each q-tile deferred past the previous tile's
normalize), remaining K/V/Q projections are interleaved into early
slots, and each q-tile's epilogue is paced through slots 11..23 of the
next tile.  Engine queues are strictly in-order, so emission order is
chosen so an instruction's dependencies are (nearly) always satisfied
when it reaches the head of its queue.
"""

import numpy as np

import concourse.bass as bass
import concourse.bacc as bacc
import concourse.mybir as mybir
import concourse.tile as tile
from concourse.bass_utils import run_bass_kernel_spmd
from concourse.masks import make_identity

F32 = mybir.dt.float32
F16 = mybir.dt.float16
I16 = mybir.dt.int16
EXP = mybir.ActivationFunctionType.Exp
IDENT = mybir.ActivationFunctionType.Identity
MUL = mybir.AluOpType.mult
ADD = mybir.AluOpType.add

B = 2
S = 4096
D = 512           # d_model
H = 8
DK = 64
HP = 4            # head pairs per batch
DL = 128          # local channels per core (2 heads)
CJ = 4            # contraction chunks of 128 over d_model
QT = S // 512     # 8 q-tiles of 512
KB = S // 128     # 32 k-blocks of 128
SCALE = 1.0 / np.sqrt(DK).item()  # 1/8
CTX_LAG = 12

# exp split: per (qt, kb) head-0's [128, 512] score tile goes to ACT
# (exact exp); head-1's [128, 512] gets a PAIR of Schraudolph bit-trick
# samples (i16 and i16+512).  The pair average -- exp(x) ~ fp16(i16) +
# 2^-0.5 * fp16(i16+512) -- is folded into the ctx matmul by
# accumulating both samples, the second against a 2^-0.5-scaled copy of
# V.  The softmax ratio cancels the overall scale.  GPSIMD cannot touch
# PSUM, so DVE computes all first samples (PSUM reads) while the int16
# +512 second samples (SBUF->SBUF) are split Pool | DVE.
ADD_DVE = 0       # head-1 +512 columns computed on DVE (rest on Pool)
# Schraudolph constants (fp16 bit layout), scale 1/8 folded into the slope
SCH_A = (1024.0 / np.log(2.0)).item() * SCALE
SCH_B = 15.0 * 1024.0 - 43.0
SCH_W = (2.0 ** -0.5)

TRACE = False
LAST_RESULTS = None
USE_SCHRAUDOLPH = True   # bisect flag: False -> exact ACT exp for head 1 too

_prog_cache = {}


def _emit(nc, reps=1):
    xqT = nc.declare_dram_parameter("xqT", [D, S], F16, isOutput=False)
    xkT = nc.declare_dram_parameter("xkT", [D, S], F16, isOutput=False)
    xvT = nc.declare_dram_parameter("xvT", [D, S], F16, isOutput=False)
    wq = nc.declare_dram_parameter("wq", [D, DL], F16, isOutput=False)
    wk = nc.declare_dram_parameter("wk", [D, DL], F16, isOutput=False)
    wv = nc.declare_dram_parameter("wv", [D, DL], F16, isOutput=False)
    bq = nc.declare_dram_parameter("bq", [DL, 1], F32, isOutput=False)
    bk = nc.declare_dram_parameter("bk", [DL, 1], F32, isOutput=False)
    bv = nc.declare_dram_parameter("bv", [DL, 1], F32, isOutput=False)
    wo = nc.declare_dram_parameter("wo", [DL, D], F16, isOutput=False)
    outT = nc.declare_dram_parameter("outT", [D, S], F16, isOutput=True)

    with tile.TileContext(nc) as tc:
        with (
            nc.allow_low_precision(reason="fp16 matmuls + schraudolph exp"),
            tc.tile_pool(name="singles", bufs=1) as singles,
            tc.tile_pool(name="xin", bufs=8) as xin,
            tc.tile_pool(name="proj", bufs=1) as proj,
            tc.tile_pool(name="pt", bufs=16) as ptpool,
            tc.tile_pool(name="sad", bufs=16) as sadpool,
            tc.tile_pool(name="ctxn", bufs=3) as ctxnpool,
            tc.tile_pool(name="ctxT", bufs=3) as ctxTpool,
            tc.tile_pool(name="outp", bufs=6) as outpool,
            tc.tile_pool(name="norm", bufs=4) as normpool,
            tc.tile_pool(name="mm512", bufs=2, space="PSUM") as mmps,
            tc.tile_pool(name="spsa", bufs=2, space="PSUM") as spsumA,
            tc.tile_pool(name="spsb", bufs=2, space="PSUM") as spsumB,
            tc.tile_pool(name="cps", bufs=2, space="PSUM") as cpsum,
        ):
            # --- constants / weights ---
            ident = singles.tile([128, 128], F16, tag="ident")
            make_identity(nc, ident)
            warmsrc = singles.tile([1, 8], F32, tag="warmsrc")
            nc.vector.memset(warmsrc, 1.0)
            warm = singles.tile([1, 8], F32, tag="warm")
            nc.scalar.activation(warm, warmsrc, EXP, scale=1.0)

            w_sb = {}
            b_sb = {}

            def load_w(name, w, bias):
                t = singles.tile([128, CJ, DL], F16, tag=name)
                nc.sync.dma_start(
                    out=t, in_=w[:].rearrange("(j p) d -> p j d", p=128)
                )
                w_sb[name] = t
                bt = singles.tile([DL, 1], F32, tag="b" + name[1])
                nc.sync.dma_start(out=bt, in_=bias[:])
                b_sb["b" + name[1]] = bt

            # --- projections: dst = (x @ W + b).T, channel-major [128, S] ---
            qpT = proj.tile([DL, S], F16, tag="qpT")
            kpT = proj.tile([DL, S], F16, tag="kpT")
            vpT = proj.tile([DL, S], F16, tag="vpT")

            # V in key-major layout: [key 128, kb 32, 130] where per head h
            # cols [65h, 65h+64) are channels and col 65h+64 is ones (the
            # softmax denominator comes out of the ctx matmul for free).
            # vp_w is head-1's slice scaled by 2^-0.5 for the second
            # Schraudolph sample's ctx accumulation.
            vp = proj.tile([128, KB, 130], F16, tag="vp")
            nc.vector.memset(vp, 1.0)
            vp_w = proj.tile([128, KB, 65], F16, tag="vp_w")

            def project_st(xT, wname, dst, st, evac_eng):
                """Columns [st*512, st*512+512) of dst = (x @ W + b).T"""
                c0 = st * 512
                xTr = xT[:].rearrange("(j p) s -> p j s", p=128)
                xt = xin.tile([128, CJ, 512], F16, tag="xin")
                nc.sync.dma_start(out=xt, in_=xTr[:, :, c0 : c0 + 512])
                ps = mmps.tile([128, 512], F32, tag="mm512")
                for cj in range(CJ):
                    nc.tensor.matmul(
                        ps,
                        lhsT=w_sb[wname][:, cj, :],
                        rhs=xt[:, cj, :],
                        start=(cj == 0),
                        stop=(cj == CJ - 1),
                    )
                bias = b_sb["b" + wname[1]]
                if evac_eng == "act":
                    nc.scalar.activation(
                        dst[:, c0 : c0 + 512], ps, IDENT, bias=bias, scale=1.0
                    )
                else:
                    nc.vector.tensor_scalar_add(dst[:, c0 : c0 + 512], ps, bias)

            def v_transpose_st(st):
                """Vp key-major blocks for the 4 k-blocks of one s-tile
                (PE transpose via identity, then per-head evac copies)."""
                ks = slice(st * 4, (st + 1) * 4)
                for kb in range(st * 4, (st + 1) * 4):
                    tp = mmps.tile([128, 512], F32, tag="mm512")
                    tpf = tp.bitcast(F16)
                    nc.tensor.transpose(
                        tpf[:, 0:128], vpT[:, kb * 128 : (kb + 1) * 128], ident
                    )
                    nc.scalar.copy(vp[:, kb, 0:64], tpf[:, 0:64])
                    nc.vector.tensor_copy(vp[:, kb, 65:129], tpf[:, 64:128])
                nc.gpsimd.tensor_scalar(
                    out=vp_w[:, ks, :],
                    in0=vp[:, ks, 65:130],
                    scalar1=SCH_W,
                    scalar2=None,
                    op0=MUL,
                )

            # Prologue: weights + the projections needed before slot (0, 0).
            # Everything else is interleaved into the attention slot stream.
            load_w("wq", wq, bq)
            project_st(xqT, "wq", qpT, 0, "act")
            load_w("wk", wk, bk)
            load_w("wv", wv, bv)
            wo_sb = singles.tile([DL, D], F16, tag="wo")
            nc.sync.dma_start(out=wo_sb, in_=wo[:])
            project_st(xkT, "wk", kpT, 0, "act")
            project_st(xvT, "wv", vpT, 0, "dve")
            v_transpose_st(0)
            project_st(xkT, "wk", kpT, 1, "act")
            project_st(xvT, "wv", vpT, 1, "dve")
            v_transpose_st(1)

            # --- attention + output projection, per q-tile of 512 ---
            # Epilogue work (normalize + transpose + Wo proj) for q-tile qt
            # is emitted piecewise during q-tile qt+1's kb loop.
            state = {}

            def norm_recip(qt, cps_h):
                zs = normpool.tile([128, 2, 4], F32, tag="zs", name="zs")
                rec = normpool.tile([128, 2, 4], F32, tag="rec", name="rec")
                state["rec"] = rec
                for h in (0, 1):
                    nc.vector.tensor_copy(zs[:, h, :], cps_h[h][:, :, 64])
                nc.vector.reciprocal(rec[:, :, :], zs[:, :, :])

            def norm_mul_one(cps_h, h, qs):
                nc.vector.tensor_scalar(
                    out=state["ctxn"][:, qs, 64 * h : 64 * h + 64],
                    in0=cps_h[h][:, qs, 0:64],
                    scalar1=state["rec"][:, h, qs : qs + 1],
                    scalar2=None,
                    op0=MUL,
                )

            def norm_mul_h(qt, cps_h, h):
                # GPSIMD cannot read PSUM; DVE multiplies by 1/Z via the
                # per-partition tensor_scalar operand.
                if h == 0:
                    state["ctxn"] = ctxnpool.tile(
                        [128, 4, 128], F16, tag="ctxn", name="ctxn"
                    )
                for qs in range(4):
                    norm_mul_one(cps_h, h, qs)

            def ctxt_qs(qt, qs):
                if qs == 0:
                    state["ctxT"] = ctxTpool.tile(
                        [128, 512], F16, tag="ctxT", name="ctxT"
                    )
                tp = mmps.tile([128, 512], F32, tag="mm512")
                tpf = tp.bitcast(F16)
                nc.tensor.transpose(tpf[:, 0:128], state["ctxn"][:, qs, :], ident)
                nc.scalar.copy(
                    state["ctxT"][:, qs * 128 : (qs + 1) * 128], tpf[:, 0:128]
                )

            def oproj_j(qt, j):
                qsl = slice(qt * 512, (qt + 1) * 512)
                ops = mmps.tile([128, 512], F32, tag="mm512")
                nc.tensor.matmul(
                    ops,
                    lhsT=wo_sb[:, j * 128 : (j + 1) * 128],
                    rhs=state["ctxT"],
                    start=True,
                    stop=True,
                )
                ot = outpool.tile([128, 512], F16, tag="out")
                nc.scalar.copy(ot, ops)
                nc.sync.dma_start(out=outT[j * 128 : (j + 1) * 128, qsl], in_=ot)

            EPI = CTX_LAG - 1  # first safe slot: last lagged ctx emitted

            def epilogue_step(step, qt, cps_h):
                s = step - EPI
                if s == 0:
                    norm_recip(qt, cps_h)
                elif s in (1, 2):
                    norm_mul_h(qt, cps_h, s - 1)
                elif 3 <= s <= 6:
                    ctxt_qs(qt, s - 3)
                elif 7 <= s <= 10:
                    oproj_j(qt, s - 7)

            def scores_exp(qt, kb):
                qsl = slice(qt * 512, (qt + 1) * 512)
                spa = spsumA.tile([128, 512], F32, tag="spsa")
                spb = spsumB.tile([128, 512], F32, tag="spsb")
                for h, sp in ((0, spa), (1, spb)):
                    nc.tensor.matmul(
                        sp,
                        lhsT=kpT[h * 64 : (h + 1) * 64, kb * 128 : (kb + 1) * 128],
                        rhs=qpT[h * 64 : (h + 1) * 64, qsl],
                        start=True,
                        stop=True,
                    )
                # head 0: exact exp on ACT
                pt = ptpool.tile([128, 512], F16, tag="pt")
                nc.scalar.activation(pt, spa, EXP, scale=SCALE)
                # head 1: two Schraudolph samples (i16, i16+512)
                sad = sadpool.tile([128, 2, 512], I16, tag="sad")
                if USE_SCHRAUDOLPH:
                    nc.vector.tensor_scalar(
                        out=sad[:, 0, :],
                        in0=spb,
                        scalar1=SCH_A,
                        scalar2=SCH_B,
                        op0=MUL,
                        op1=ADD,
                    )
                    if ADD_DVE > 0:
                        nc.vector.tensor_scalar_add(
                            sad[:, 1, 0:ADD_DVE], sad[:, 0, 0:ADD_DVE], 512.0
                        )
                    nc.gpsimd.tensor_scalar_add(
                        sad[:, 1, ADD_DVE:512], sad[:, 0, ADD_DVE:512], 512.0
                    )
                else:
                    nc.scalar.activation(
                        sad.bitcast(F16)[:, 0, :], spb, EXP, scale=SCALE
                    )
                    nc.vector.memset(sad[:, 1, :], 0)
                return (pt, sad)

            def ctx_mm(cps_h, kb, ptt):
                pt, sad = ptt
                # One PSUM accumulation group per head per q-tile (a PSUM
                # zero region is a whole bank): start only on the very first
                # matmul, stop on the very last.
                # head 0: plain accumulation of the exact-exp tile
                for qs in range(4):
                    nc.tensor.matmul(
                        cps_h[0][:, qs, 0:65],
                        lhsT=pt[:, qs * 128 : (qs + 1) * 128],
                        rhs=vp[:, kb, 0:65],
                        start=(kb == 0 and qs == 0),
                        stop=(kb == KB - 1 and qs == 3),
                        skip_group_check=True,
                    )
                # head 1: both Schraudolph samples; sample B against the
                # 2^-0.5-scaled V copy (pair-average folded into the sum)
                for qs in range(4):
                    blk = lambda smp: sad.bitcast(F16)[
                        :, smp, qs * 128 : (qs + 1) * 128
                    ]
                    for smp, rhs in ((0, vp[:, kb, 65:130]), (1, vp_w[:, kb, :])):
                        nc.tensor.matmul(
                            cps_h[1][:, qs, 0:65],
                            lhsT=blk(smp),
                            rhs=rhs,
                            start=(kb == 0 and qs == 0 and smp == 0),
                            stop=(kb == KB - 1 and qs == 3 and smp == 1),
                            skip_group_check=True,
                        )

            # Flat software pipeline over all (qt, kb) slots: ctx lags
            # CTX_LAG slots globally (crossing q-tile boundaries).  The
            # first 4 ctx batches of each q-tile are deferred to slot
            # kb=14 so the previous q-tile's normalize (slots 11-13) has
            # released the cps buffers before the start=True write.  The
            # remaining K/V/Q projections stream through early slots so
            # the PE never sees a serial projection phase.
            slots = [
                (qt, kb)
                for _ in range(reps)
                for qt in range(QT)
                for kb in range(KB)
            ]
            nslots = len(slots)
            cps_of = {}
            pts = {}
            deferred = {}
            pending = None  # (qt, cps_h) awaiting epilogue

            def emit_ctx(j):
                jqt, jkb = slots[j]
                if jkb == 0:
                    cps0 = cpsum.tile([128, 4, 128], F32, tag="cps", name="cps0")
                    cps1 = cpsum.tile([128, 4, 128], F32, tag="cps", name="cps1")
                    cps_of[jqt] = (cps0, cps1)
                ctx_mm(cps_of[jqt], jkb, pts.pop(j))

            for i, (qt, kb) in enumerate(slots):
                j = i - CTX_LAG
                if kb == CTX_LAG + 2 and qt in deferred:
                    for j2 in deferred.pop(qt):
                        emit_ctx(j2)
                if j >= 0:
                    jqt, jkb = slots[j]
                    if jkb < 2:
                        deferred.setdefault(jqt, []).append(j)
                    else:
                        emit_ctx(j)
                pts[i] = scores_exp(qt, kb)
                # interleaved projections (first pass only), two s-tiles
                # of lookahead so the evac/transpose latency is covered
                if qt == 0 and kb % 4 == 0 and 1 < (st := kb // 4 + 2) < QT:
                    project_st(xkT, "wk", kpT, st, "act")
                    project_st(xvT, "wv", vpT, st, "dve")
                    v_transpose_st(st)
                if kb == 20 and qt + 1 < QT:
                    project_st(xqT, "wq", qpT, qt + 1, "dve")
                if pending is not None:
                    epilogue_step(kb, *pending)
                if kb == KB - 1:
                    pending = (qt, cps_of[qt])
            for j in range(nslots - CTX_LAG, nslots):
                emit_ctx(j)
            # Final epilogue: qs-major so each transpose starts as soon as
            # its own two normalize muls land, instead of after all eight.
            fqt, fcps = pending
            norm_recip(fqt, fcps)
            state["ctxn"] = ctxnpool.tile(
                [128, 4, 128], F16, tag="ctxn", name="ctxnf"
            )
            for qs in range(4):
                norm_mul_one(fcps, 0, qs)
                norm_mul_one(fcps, 1, qs)
                ctxt_qs(fqt, qs)
            for j in range(4):
                oproj_j(fqt, j)
    return nc


def _build(reps=1):
    if reps not in _prog_cache:
        nc = bacc.Bacc()
        _emit(nc, reps)
        nc.compile()
        _prog_cache[reps] = nc
    return _prog_cache[reps]


def _make_in_maps(query, key, value, Wq, bq, Wk, bk, Wv, bv, Wo):
    f16 = lambda a: np.ascontiguousarray(a, dtype=np.float16)
    f32 = lambda a: np.ascontiguousarray(a, dtype=np.float32)
    in_maps = []
    for b in range(B):
        xqT = f16(query[b].T)
        xkT = f16(key[b].T)
        xvT = f16(value[b].T)
        for hp in range(HP):
            cs = slice(hp * DL, (hp + 1) * DL)
            in_maps.append(
                {
                    "xqT": xqT,
                    "xkT": xkT,
                    "xvT": xvT,
                    "wq": f16(Wq[:, cs]),
                    "wk": f16(Wk[:, cs]),
                    "wv": f16(Wv[:, cs]),
                    "bq": f32(bq[cs].reshape(DL, 1)),
                    "bk": f32(bk[cs].reshape(DL, 1)),
                    "bv": f32(bv[cs].reshape(DL, 1)),
                    "wo": f16(Wo[cs, :]),
                }
            )
    return in_maps


def kernel(query, key, value, Wq, bq, Wk, bk, Wv, bv, Wo, bo):
    global LAST_RESULTS
    query = np.asarray(query, dtype=np.float32)
    key = np.asarray(key, dtype=np.float32)
    value = np.asarray(value, dtype=np.float32)
    Wq = np.asarray(Wq, dtype=np.float32)
    Wk = np.asarray(Wk, dtype=np.float32)
    Wv = np.asarray(Wv, dtype=np.float32)
    Wo = np.asarray(Wo, dtype=np.float32)
    bq = np.asarray(bq, dtype=np.float32)
    bk = np.asarray(bk, dtype=np.float32)
    bv = np.asarray(bv, dtype=np.float32)
    bo = np.asarray(bo, dtype=np.float32)

    nc = _build()
    in_maps = _make_in_maps(query, key, value, Wq, bq, Wk, bk, Wv, bv, Wo)

    res = run_bass_kernel_spmd(nc, in_maps, list(range(B * HP)), trace=TRACE)
    LAST_RESULTS = res

    out = np.empty((B, S, D), dtype=np.float32)
    for b in range(B):
        acc = res.results[b * HP]["outT"].astype(np.float32)
        for hp in range(1, HP):
            acc = acc + res.results[b * HP + hp]["outT"].astype(np.float32)
        out[b] = acc.T + bo
    return out
